# revision 5
# baseline (speedup 1.0000x reference)
"""AdaptiveCornerLoss on 8 TRN2 NeuronCores — batch-parallel Bass/Tile kernel.

Shapes (hardcoded): B=64, N=16384, C=6, M=128 corners. 8 cores, 8 samples/core.

Math:
  focal    = u^2 * ce  with  y=(1-2t)*x, ce=softplus(y)=ln(1+e^y),
             u=sigmoid(y)  =>  u^2 = exp(-2*ln(1+e^{-y}))
  d2(n,m)  = |p|^2 + |c|^2 - 2 p.c   (augmented fp16 matmul; per-point feature
             rows [px,py,pz,|p|^2,1] vs corner rows [-2cx,-2cy,-2cz,1,|c|^2+pen])
  w        = exp(-10*sqrt(max(min_m d2, 1e-12))) via exp/ln only (one ACT set)

Layout/engine tricks:
  * Valid corners host-compacted per sample; kernel built for Mk =
    roundup(max valid, 32) corners (96 on the graded data). Padding corners
    carry |c|^2+pen so they never win the min.
  * Pairwise-min offload: corners paired (2j, 2j+1). PE emits, per chunk,
    A = d2 vs even corners [Mh cols] and E = d2_even - d2_odd [Mh cols]
    (difference features are linear -> one matmul). ACT computes R=relu(E)
    straight out of PSUM; DVE computes pair-min = A - R (bf16) and a 2x-mode
    bf16 min tree. This splits the PSUM drain evenly between ACT and DVE and
    halves the DVE tree input.
  * CPG=4 chunks share one LDWEIGHTS (stationary K=20 stacks 4 chunks'
    features; the 4 zero-padded rhs variants are adjacent so ONE matmul per
    group computes all 4 chunks: 256 LDW+MM pairs total, not 1024).
  * PSUM tiles span 4 banks; each bank = one group's [128, 4*Mk] grid.
Outputs per core: per-partition partial sums [128,2] of (focal, focal*w);
host reduces and forms (total, focal_loss, distance_loss).
"""

import sys

sys.path.insert(0, "/opt/trn_rl_repo")
sys.path.insert(0, "/root/problem")

import numpy as np

import concourse.bass as bass
import concourse.mybir as mybir
from concourse import tile
from concourse.bass_utils import run_bass_kernel_spmd
from waitsplit import split_waits

NCORES = 8
B, N, M = 64, 16384, 128
S = B // NCORES          # samples per core
K = 5                    # feature rows per chunk
CPG = 4                  # chunks sharing one LDWEIGHTS group / one matmul
CH = N // 128            # 128-point chunks per sample (128)
GRP = CH // CPG          # groups per sample (32)
TPS = 8                  # psum tiles per sample (4 groups = 16 chunks each)
COLS = S * CH            # minsq/logit columns per core (1024)
PEN = 100.0

F = mybir.ActivationFunctionType
OP = mybir.AluOpType
DT = mybir.dt

_CACHE = {}


def build_nc(Mk):
    Mh = Mk // 2
    nc = bass.Bass()
    lhsT = nc.declare_dram_parameter(
        "lhsT", [S, K * CPG, N // CPG], DT.float16, isOutput=False
    )
    rhs = nc.declare_dram_parameter(
        "rhs", [K * CPG, S * CPG * Mk], DT.float16, isOutput=False
    )
    lg = nc.declare_dram_parameter("lg", [128, COLS], DT.float32, isOutput=False)
    tg = nc.declare_dram_parameter("tg", [128, COLS], DT.float32, isOutput=False)
    out = nc.declare_dram_parameter("out", [128, 2], DT.float32, isOutput=True)

    # bf16 min-tree levels: Mh -> ... -> wlast (tensor_reduce finishes)
    levels = []
    w = Mh
    while w % 2 == 0 and w > 6:
        w //= 2
        levels.append(w)

    with tile.TileContext(nc) as tc:
        with (
            tc.tile_pool(name="persist", bufs=1) as pp,
            tc.tile_pool(name="stream", bufs=2) as wp,
            tc.tile_pool(name="psum", bufs=2, space="PSUM") as psp,
        ):
            # --- resident inputs
            rt = pp.tile([K * CPG, S * CPG * Mk], DT.float16)
            nc.sync.dma_start(out=rt[:], in_=rhs[:])
            lgt = pp.tile([128, COLS], DT.float32)
            nc.sync.dma_start(out=lgt[:], in_=lg[:])
            tgt = pp.tile([128, COLS], DT.float32)
            nc.sync.dma_start(out=tgt[:], in_=tg[:])

            sums = pp.tile([128, 2], DT.float32)

            # --- focal chain (mostly ACT; overlaps the grid work below)
            y = pp.tile([128, COLS], DT.float32)
            ce = pp.tile([128, COLS], DT.float32)
            u2 = pp.tile([128, COLS], DT.float32)
            fo = pp.tile([128, COLS], DT.float32)
            nc.vector.tensor_scalar(
                out=y[:], in0=tgt[:], scalar1=-2.0, scalar2=1.0, op0=OP.mult, op1=OP.add
            )
            nc.vector.tensor_tensor(out=y[:], in0=y[:], in1=lgt[:], op=OP.mult)
            nc.scalar.activation(ce[:], y[:], F.Exp)                  # e^y
            nc.scalar.activation(ce[:], ce[:], F.Ln, bias=1.0)        # ce = ln(1+e^y)
            nc.scalar.activation(u2[:], y[:], F.Exp, scale=-1.0)      # e^-y
            nc.scalar.activation(u2[:], u2[:], F.Ln, bias=1.0)        # ln(1+e^-y)
            nc.scalar.activation(u2[:], u2[:], F.Exp, scale=-2.0)     # u^2
            nc.vector.tensor_tensor(out=fo[:], in0=ce[:], in1=u2[:], op=OP.mult)
            nc.vector.tensor_reduce(
                out=sums[:, 0:1], in_=fo[:], axis=mybir.AxisListType.X, op=OP.add
            )

            # --- distance grid
            minsq = pp.tile([128, COLS], DT.float32)
            for s in range(S):
                lt = wp.tile([K * CPG, N // CPG], DT.float16, tag="lhsT")
                nc.sync.dma_start(out=lt[:], in_=lhsT[s])
                for tp in range(TPS // 2):  # process psum tiles in pairs
                    trb = wp.tile([128, 32 * Mh], DT.bfloat16, tag="tree0")
                    for half in range(2):
                        t = tp * 2 + half
                        pt = psp.tile([128, 2048], DT.float32)  # 4 banks, 16 chunks
                        for bk in range(4):
                            g = t * 4 + bk
                            nc.tensor.matmul(
                                out=pt[:, 512 * bk: 512 * bk + CPG * Mk],
                                lhsT=lt[:, g * 128:(g + 1) * 128],
                                rhs=rt[:, s * CPG * Mk:(s + 1) * CPG * Mk],
                                start=True, stop=True,
                            )
                        grid = pt[:].rearrange("p (b r) -> p b r", r=512)
                        grid = grid[:, :, 0: CPG * Mk]
                        grid = grid.rearrange("p b (v m) -> p b v m", m=Mk)
                        # R = relu(E) out of PSUM (ACT), pair-min = A - R (DVE)
                        rl = wp.tile([128, 16 * Mh], DT.float32, tag="relu")
                        nc.scalar.activation(rl[:], grid[:, :, :, Mh:Mk], F.Relu)
                        nc.vector.tensor_tensor(
                            out=trb[:, half * 16 * Mh:(half + 1) * 16 * Mh],
                            in0=grid[:, :, :, 0:Mh],
                            in1=rl[:].rearrange("p (b v m) -> p b v m", v=CPG, m=Mh),
                            op=OP.subtract,
                        )
                    # bf16 2x min tree over 32 chunks
                    cur = trb[:].rearrange("p (c m) -> p c m", m=Mh)
                    width = Mh
                    for wnext in levels:
                        nxt = wp.tile([128, 32 * wnext], DT.bfloat16,
                                      tag=f"tree{wnext}")
                        nc.vector.tensor_tensor(
                            out=nxt[:].rearrange("p (c m) -> p c m", m=wnext),
                            in0=cur[:, :, 0:wnext],
                            in1=cur[:, :, wnext:2 * wnext],
                            op=OP.min,
                        )
                        cur = nxt[:].rearrange("p (c m) -> p c m", m=wnext)
                        width = wnext
                    c0 = s * CH + tp * 32
                    nc.vector.tensor_reduce(
                        out=minsq[:, c0:c0 + 32],
                        in_=cur,
                        axis=mybir.AxisListType.X,
                        op=OP.min,
                    )

            # --- epilogue: w = exp(-10*sqrt(max(minsq,1e-12))), S2 = sum(fo*w)
            nc.vector.tensor_scalar_max(out=minsq[:], in0=minsq[:], scalar1=1e-12)
            nc.scalar.activation(minsq[:], minsq[:], F.Ln)
            nc.scalar.activation(minsq[:], minsq[:], F.Exp, scale=0.5)    # sqrt
            nc.scalar.activation(minsq[:], minsq[:], F.Exp, scale=-10.0)  # w
            nc.vector.tensor_tensor(out=y[:], in0=fo[:], in1=minsq[:], op=OP.mult)
            nc.vector.tensor_reduce(
                out=sums[:, 1:2], in_=y[:], axis=mybir.AxisListType.X, op=OP.add
            )
            nc.sync.dma_start(out=out[:], in_=sums[:])

    split_waits(nc)
    return nc


def pack_inputs(inputs, targets, point_coords, corner_coords):
    """Host-side shard + layout packing. Returns (in_maps, Mk)."""
    x = np.asarray(inputs, np.float32)
    t = np.asarray(targets, np.float32)
    pc = np.asarray(point_coords, np.float32)
    cc = np.asarray(corner_coords, np.float32)

    pts = pc[..., :3]
    q = (pts * pts).sum(-1)
    feats = np.empty((B, K, N), np.float32)
    feats[:, 0] = pts[..., 0]
    feats[:, 1] = pts[..., 1]
    feats[:, 2] = pts[..., 2]
    feats[:, 3] = q
    feats[:, 4] = 1.0
    # [B, K, GRP, CPG, 128] -> [B, CPG, K, GRP, 128] -> [B, K*CPG, GRP*128]
    fg = feats.reshape(B, K, GRP, CPG, 128).transpose(0, 3, 1, 2, 4)
    lhsT = fg.reshape(B, K * CPG, N // CPG).astype(np.float16)

    # corners: compact valid to front, pad with PEN sentinels at origin
    valid = cc[..., 0] > -1.0
    nv = valid.sum(-1)
    maxv = int(nv.max()) if nv.max() > 0 else 1
    Mk = min(M, ((maxv + 31) // 32) * 32)
    Mh = Mk // 2
    cfeat = np.zeros((B, K, Mk), np.float32)
    cfeat[:, 4] = PEN
    for b in range(B):
        v = cc[b][valid[b]]
        n = v.shape[0]
        cfeat[b, 0, :n] = -2.0 * v[:, 0]
        cfeat[b, 1, :n] = -2.0 * v[:, 1]
        cfeat[b, 2, :n] = -2.0 * v[:, 2]
        cfeat[b, 3, :n] = 1.0
        cfeat[b, 4, :n] = (v * v).sum(-1)
    # pairwise: A features (even corners), E features (even - odd)
    fA = cfeat[:, :, 0::2]                       # [B, K, Mh]
    fE = fA - cfeat[:, :, 1::2]                  # [B, K, Mh]
    blk = np.concatenate([fA, fE], axis=2)       # [B, K, Mk]: [A | E]
    rhs = np.zeros((B, CPG, K * CPG, Mk), np.float32)
    for v in range(CPG):
        rhs[:, v, v * K:(v + 1) * K, :] = blk
    rhs = rhs.astype(np.float16)

    in_maps = []
    for c in range(NCORES):
        sl = slice(c * S, (c + 1) * S)
        lgp = x[sl].reshape(S, CH, 128).transpose(2, 0, 1).reshape(128, COLS).copy()
        tgp = t[sl].reshape(S, CH, 128).transpose(2, 0, 1).reshape(128, COLS).copy()
        rhp = rhs[sl].transpose(2, 0, 1, 3).reshape(K * CPG, S * CPG * Mk).copy()
        in_maps.append({
            "lhsT": np.ascontiguousarray(lhsT[sl]),
            "rhs": rhp,
            "lg": lgp,
            "tg": tgp,
        })
    return in_maps, Mk


def _finalize(results):
    s1 = 0.0
    s2 = 0.0
    for r in results:
        o = np.asarray(r["out"], np.float64)
        s1 += o[:, 0].sum()
        s2 += o[:, 1].sum()
    bn = float(B * N)
    focal = s1 / bn
    distance = (s1 + 2.0 * s2) / bn
    total = focal + distance
    return (np.float32(total), np.float32(focal), np.float32(distance))


def kernel(inputs, targets, point_coords, corner_coords):
    in_maps, Mk = pack_inputs(inputs, targets, point_coords, corner_coords)
    if Mk not in _CACHE:
        _CACHE[Mk] = build_nc(Mk)
    nc = _CACHE[Mk]
    res = run_bass_kernel_spmd(nc, in_maps, core_ids=list(range(NCORES)))
    return _finalize(res.results)


if __name__ == "__main__":
    rng = np.random.default_rng(0)
    ins = {
        "inputs": rng.standard_normal((B, N), dtype=np.float32),
        "targets": (rng.random((B, N)) < 0.05).astype(np.float32),
        "point_coords": rng.random((B, N, 6), dtype=np.float32),
        "corner_coords": rng.random((B, 128, 3), dtype=np.float32),
    }
    print(kernel(**ins))


# revision 9
# speedup vs baseline: 1.1021x; 1.1021x over previous
"""AdaptiveCornerLoss on 8 TRN2 NeuronCores — batch-parallel Bass/Tile kernel.

Shapes (hardcoded): B=64, N=16384, C=6, M=128 corners. 8 cores, 8 samples/core.

Math:
  focal    = u^2 * ce  with  y=(1-2t)*x, ce=softplus(y)=ln(1+e^y),
             u=sigmoid(y)  =>  u^2 = exp(-2*ln(1+e^{-y}))
  d2(n,m)  = |p|^2 + |c|^2 - 2 p.c   (augmented fp16 matmul; per-point feature
             rows [px,py,pz,|p|^2,1] vs corner rows [-2cx,-2cy,-2cz,1,|c|^2+pen])
  w        = exp(-10*sqrt(max(min_m d2, 1e-12))) via exp/ln only (one ACT set)

Layout/engine tricks:
  * Valid corners host-compacted per sample; kernel built for Mk =
    roundup(max valid, 32) corners (96 on the graded data). Padding corners
    carry |c|^2+pen so they never win the min.
  * Pairwise-min offload: corners paired (2j, 2j+1). PE emits, per chunk,
    A = d2 vs even corners [Mh cols] and E = d2_even - d2_odd [Mh cols]
    (difference features are linear -> one matmul). ACT computes R=relu(E)
    straight out of PSUM; DVE computes pair-min = A - R (bf16) and a 2x-mode
    bf16 min tree. This splits the PSUM drain evenly between ACT and DVE and
    halves the DVE tree input.
  * CPG=4 chunks share one LDWEIGHTS (stationary K=20 stacks 4 chunks'
    features; the 4 zero-padded rhs variants are adjacent so ONE matmul per
    group computes all 4 chunks: 256 LDW+MM pairs total, not 1024).
  * PSUM tiles span 4 banks; each bank = one group's [128, 4*Mk] grid.
Outputs per core: per-partition partial sums [128,2] of (focal, focal*w);
host reduces and forms (total, focal_loss, distance_loss).
"""

import sys

sys.path.insert(0, "/opt/trn_rl_repo")
sys.path.insert(0, "/root/problem")

import numpy as np

import concourse.bass as bass
import concourse.mybir as mybir
from concourse import tile
from concourse.bass_utils import run_bass_kernel_spmd
from waitsplit import split_waits

NCORES = 8
B, N, M = 64, 16384, 128
S = B // NCORES          # samples per core
K = 5                    # feature rows per chunk
CPG = 4                  # chunks sharing one LDWEIGHTS group / one matmul
CH = N // 128            # 128-point chunks per sample (128)
GRP = CH // CPG          # groups per sample (32)
TPS = 8                  # psum tiles per sample (4 groups = 16 chunks each)
COLS = S * CH            # minsq/logit columns per core (1024)
PEN = 100.0

F = mybir.ActivationFunctionType
OP = mybir.AluOpType
DT = mybir.dt

_CACHE = {}


def build_nc(Mk):
    Mh = Mk // 2
    nc = bass.Bass()
    lhsT = nc.declare_dram_parameter(
        "lhsT", [S, K * CPG, N // CPG], DT.float16, isOutput=False
    )
    rhs = nc.declare_dram_parameter(
        "rhs", [K * CPG, S * CPG * Mk], DT.float16, isOutput=False
    )
    lg = nc.declare_dram_parameter("lg", [128, COLS], DT.float32, isOutput=False)
    tg = nc.declare_dram_parameter("tg", [128, COLS], DT.float32, isOutput=False)
    out = nc.declare_dram_parameter("out", [128, 2], DT.float32, isOutput=True)

    # bf16 min-tree levels: Mh -> ... -> wlast (tensor_reduce finishes)
    levels = []
    w = Mh
    while w % 2 == 0 and w > 6:
        w //= 2
        levels.append(w)

    with tile.TileContext(nc) as tc:
        with (
            tc.tile_pool(name="persist", bufs=1) as pp,
            tc.tile_pool(name="stream", bufs=2) as wp,
            tc.tile_pool(name="relupool", bufs=4) as rp,
            tc.tile_pool(name="psum", bufs=4, space="PSUM") as psp,
        ):
            # --- resident inputs
            rt = pp.tile([K * CPG, S * CPG * Mk], DT.float16)
            nc.sync.dma_start(out=rt[:], in_=rhs[:])
            lgt = pp.tile([128, COLS], DT.float32)
            nc.sync.dma_start(out=lgt[:], in_=lg[:])
            tgt = pp.tile([128, COLS], DT.float32)
            nc.sync.dma_start(out=tgt[:], in_=tg[:])

            sums = pp.tile([128, 2], DT.float32)

            # --- focal chain (mostly ACT; overlaps the grid work below)
            y = pp.tile([128, COLS], DT.float32)
            ce = pp.tile([128, COLS], DT.float32)
            u2 = pp.tile([128, COLS], DT.float32)
            fo = pp.tile([128, COLS], DT.float32)
            nc.vector.tensor_scalar(
                out=y[:], in0=tgt[:], scalar1=-2.0, scalar2=1.0, op0=OP.mult, op1=OP.add
            )
            nc.gpsimd.tensor_tensor(out=y[:], in0=y[:], in1=lgt[:], op=OP.mult)
            nc.scalar.activation(ce[:], y[:], F.Exp)                  # e^y
            nc.scalar.activation(ce[:], ce[:], F.Ln, bias=1.0)        # ce = ln(1+e^y)
            nc.scalar.activation(u2[:], y[:], F.Exp, scale=-1.0)      # e^-y
            nc.scalar.activation(u2[:], u2[:], F.Ln, bias=1.0)        # ln(1+e^-y)
            nc.scalar.activation(u2[:], u2[:], F.Exp, scale=-2.0)     # u^2
            nc.gpsimd.tensor_tensor(out=fo[:], in0=ce[:], in1=u2[:], op=OP.mult)
            nc.vector.tensor_reduce(
                out=sums[:, 0:1], in_=fo[:], axis=mybir.AxisListType.X, op=OP.add
            )

            # --- distance grid: 2-bank psum tiles (8 chunks), 4-deep pipeline
            minsq = pp.tile([128, COLS], DT.float32)
            for s in range(S):
                lt = wp.tile([K * CPG, N // CPG], DT.float16, tag="lhsT")
                nc.sync.dma_start(out=lt[:], in_=lhsT[s])
                for tp in range(4):  # 4 tree batches of 32 chunks per sample
                    trb = wp.tile([128, 32 * Mh], DT.bfloat16, tag="tree0")
                    for half in range(4):
                        t = tp * 4 + half
                        pt = psp.tile([128, 1024], DT.float32)  # 2 banks, 8 chunks
                        for bk in range(2):
                            g = t * 2 + bk
                            nc.tensor.matmul(
                                out=pt[:, 512 * bk: 512 * bk + CPG * Mk],
                                lhsT=lt[:, g * 128:(g + 1) * 128],
                                rhs=rt[:, s * CPG * Mk:(s + 1) * CPG * Mk],
                                start=True, stop=True,
                            )
                        grid = pt[:].rearrange("p (b r) -> p b r", r=512)
                        grid = grid[:, :, 0: CPG * Mk]
                        grid = grid.rearrange("p b (v m) -> p b v m", m=Mk)
                        # R = relu(E) out of PSUM (ACT), pair-min = A - R (DVE)
                        rl = rp.tile([128, 8 * Mh], DT.float32, tag="relu")
                        nc.scalar.activation(rl[:], grid[:, :, :, Mh:Mk], F.Relu)
                        nc.vector.tensor_tensor(
                            out=trb[:, half * 8 * Mh:(half + 1) * 8 * Mh],
                            in0=grid[:, :, :, 0:Mh],
                            in1=rl[:].rearrange("p (b v m) -> p b v m", v=CPG, m=Mh),
                            op=OP.subtract,
                        )
                    # bf16 2x min tree over 32 chunks
                    cur = trb[:].rearrange("p (c m) -> p c m", m=Mh)
                    width = Mh
                    for wnext in levels:
                        nxt = wp.tile([128, 32 * wnext], DT.bfloat16,
                                      tag=f"tree{wnext}")
                        nc.vector.tensor_tensor(
                            out=nxt[:].rearrange("p (c m) -> p c m", m=wnext),
                            in0=cur[:, :, 0:wnext],
                            in1=cur[:, :, wnext:2 * wnext],
                            op=OP.min,
                        )
                        cur = nxt[:].rearrange("p (c m) -> p c m", m=wnext)
                        width = wnext
                    c0 = s * CH + tp * 32
                    nc.vector.tensor_reduce(
                        out=minsq[:, c0:c0 + 32],
                        in_=cur,
                        axis=mybir.AxisListType.X,
                        op=OP.min,
                    )

            # --- epilogue: w = exp(-10*sqrt(max(minsq,1e-12))), S2 = sum(fo*w)
            nc.vector.tensor_scalar_max(out=minsq[:], in0=minsq[:], scalar1=1e-12)
            nc.scalar.activation(minsq[:], minsq[:], F.Ln)
            nc.scalar.activation(minsq[:], minsq[:], F.Exp, scale=0.5)    # sqrt
            nc.scalar.activation(minsq[:], minsq[:], F.Exp, scale=-10.0)  # w
            nc.gpsimd.tensor_tensor(out=y[:], in0=fo[:], in1=minsq[:], op=OP.mult)
            nc.vector.tensor_reduce(
                out=sums[:, 1:2], in_=y[:], axis=mybir.AxisListType.X, op=OP.add
            )
            nc.sync.dma_start(out=out[:], in_=sums[:])

    split_waits(nc)
    return nc


def pack_inputs(inputs, targets, point_coords, corner_coords):
    """Host-side shard + layout packing. Returns (in_maps, Mk)."""
    x = np.asarray(inputs, np.float32)
    t = np.asarray(targets, np.float32)
    pc = np.asarray(point_coords, np.float32)
    cc = np.asarray(corner_coords, np.float32)

    pts = pc[..., :3]
    q = (pts * pts).sum(-1)
    feats = np.empty((B, K, N), np.float32)
    feats[:, 0] = pts[..., 0]
    feats[:, 1] = pts[..., 1]
    feats[:, 2] = pts[..., 2]
    feats[:, 3] = q
    feats[:, 4] = 1.0
    # [B, K, GRP, CPG, 128] -> [B, CPG, K, GRP, 128] -> [B, K*CPG, GRP*128]
    fg = feats.reshape(B, K, GRP, CPG, 128).transpose(0, 3, 1, 2, 4)
    lhsT = fg.reshape(B, K * CPG, N // CPG).astype(np.float16)

    # corners: compact valid to front, pad with PEN sentinels at origin
    valid = cc[..., 0] > -1.0
    nv = valid.sum(-1)
    maxv = int(nv.max()) if nv.max() > 0 else 1
    Mk = min(M, ((maxv + 31) // 32) * 32)
    Mh = Mk // 2
    cfeat = np.zeros((B, K, Mk), np.float32)
    cfeat[:, 4] = PEN
    for b in range(B):
        v = cc[b][valid[b]]
        n = v.shape[0]
        cfeat[b, 0, :n] = -2.0 * v[:, 0]
        cfeat[b, 1, :n] = -2.0 * v[:, 1]
        cfeat[b, 2, :n] = -2.0 * v[:, 2]
        cfeat[b, 3, :n] = 1.0
        cfeat[b, 4, :n] = (v * v).sum(-1)
    # pairwise: A features (even corners), E features (even - odd)
    fA = cfeat[:, :, 0::2]                       # [B, K, Mh]
    fE = fA - cfeat[:, :, 1::2]                  # [B, K, Mh]
    blk = np.concatenate([fA, fE], axis=2)       # [B, K, Mk]: [A | E]
    rhs = np.zeros((B, CPG, K * CPG, Mk), np.float32)
    for v in range(CPG):
        rhs[:, v, v * K:(v + 1) * K, :] = blk
    rhs = rhs.astype(np.float16)

    in_maps = []
    for c in range(NCORES):
        sl = slice(c * S, (c + 1) * S)
        lgp = x[sl].reshape(S, CH, 128).transpose(2, 0, 1).reshape(128, COLS).copy()
        tgp = t[sl].reshape(S, CH, 128).transpose(2, 0, 1).reshape(128, COLS).copy()
        rhp = rhs[sl].transpose(2, 0, 1, 3).reshape(K * CPG, S * CPG * Mk).copy()
        in_maps.append({
            "lhsT": np.ascontiguousarray(lhsT[sl]),
            "rhs": rhp,
            "lg": lgp,
            "tg": tgp,
        })
    return in_maps, Mk


def _finalize(results):
    s1 = 0.0
    s2 = 0.0
    for r in results:
        o = np.asarray(r["out"], np.float64)
        s1 += o[:, 0].sum()
        s2 += o[:, 1].sum()
    bn = float(B * N)
    focal = s1 / bn
    distance = (s1 + 2.0 * s2) / bn
    total = focal + distance
    return (np.float32(total), np.float32(focal), np.float32(distance))


def kernel(inputs, targets, point_coords, corner_coords):
    in_maps, Mk = pack_inputs(inputs, targets, point_coords, corner_coords)
    if Mk not in _CACHE:
        _CACHE[Mk] = build_nc(Mk)
    nc = _CACHE[Mk]
    res = run_bass_kernel_spmd(nc, in_maps, core_ids=list(range(NCORES)))
    return _finalize(res.results)


if __name__ == "__main__":
    rng = np.random.default_rng(0)
    ins = {
        "inputs": rng.standard_normal((B, N), dtype=np.float32),
        "targets": (rng.random((B, N)) < 0.05).astype(np.float32),
        "point_coords": rng.random((B, N, 6), dtype=np.float32),
        "corner_coords": rng.random((B, 128, 3), dtype=np.float32),
    }
    print(kernel(**ins))
